# revision 9
# baseline (speedup 1.0000x reference)
"""Trainium2 Bass kernel for nn_CCepLTVFilter (v5).

Frequency-sharded across 8 cores (128 freqs each); every core computes all
256 (b,t) frames; per-core outputs are partial sums of the full output.

Device pipeline:
  1. Yr/Yi = G^T @ xcat, H^T @ xcat   (PE; G = w2@CF, H = w2@SF folded on
     host, so the conv stage disappears from the device critical path)
  2. mag=(1+t)/(1-t), t=tanh(Yr); sin/cos via Sin ACT — single table set
     (pinned by a dummy Silu so no mid-chain ACT table switch)
  3. Zr/Zi = DFT of frames (PE; frames pre-transposed on host)
  4. P = (mag cos + i mag sin)(Zr + iZi)   (DVE; fp16 tail products)
  5. step6+OLA fused: ob[t,h] = P@CO_l/SO_l + P(t-1)@CO_r/SO_r with the
     frame roll expressed as a shifted stationary slice (wrap column).
All matmul operands fp16, fp32 PSUM accumulation; fp16 partial outputs
summed in fp32 on host. DMAs only on sync/scalar hardware queues.
Separate PSUM tiles per result so tile-granular waits stay minimal.
PE warmup dummies bridge DMA waits and keep the HAM clock-gate open.
"""

import numpy as np

import concourse.bass as bass
import concourse.bacc as bacc
import concourse.mybir as mybir
import concourse.tile as tile
from concourse.bass_utils import run_bass_kernel_spmd

# ---------------- problem dims (hardcoded) ----------------
B, T, D = 2, 128, 80
CCEP = 222
FFT = 1024
HOP = 256
WIN = 2 * HOP            # 512
PAD = (FFT - CCEP) // 2  # 401
M = FFT + 1              # 1025-point transforms
BT = B * T               # 256
NCORES = 8
FS = FFT // NCORES       # 128 frequencies per core
LAM = float(np.log(10.0) / 10.0)

F16 = mybir.dt.float16
F32 = mybir.dt.float32
PI = float(np.pi)
AF = mybir.ActivationFunctionType
ALU = mybir.AluOpType

# bigA cols (fp16): xcatA[0:256) xcatB[256:512) GA[512:640) GB[640:768)
CA = 768
CH = 256                 # bigH: HA[0:128) HB[128:256)
CD = 1024                # bigD: co[0:512) so[512:1024)
CE = 2048                # bigE: fr[0:1024) zc[1024:1536) zs[1536:2048)

TRACE = False
LAST_RESULT = None


# ---------------- host-side constants (input independent) ----------------
def _make_constants():
    o = np.arange(CCEP, dtype=np.float64)[:, None]
    f = np.arange(FFT, dtype=np.float64)[None, :]
    qn_idx = np.arange(1, CCEP // 2 + 1, dtype=np.float64)
    qnorm = np.concatenate([qn_idx[::-1], qn_idx])
    ang = 2.0 * np.pi * f * (o + PAD) / FFT
    CF = np.cos(ang) * (LAM / 2.0) / qnorm[:, None]      # [222,1024]
    SF = -np.sin(ang) / qnorm[:, None]

    u = np.arange(WIN, dtype=np.float64)[:, None]
    phi = 2.0 * np.pi * f * (u + FFT // 2) / M
    ZC = np.cos(phi)                                     # [512,1024]
    ZS = np.sin(phi)

    w = np.arange(WIN, dtype=np.float64)[None, :]
    th = 2.0 * np.pi * np.arange(FFT, dtype=np.float64)[:, None] * w / M
    win = 0.5 * (1.0 - np.cos(2.0 * np.pi * np.arange(WIN) / WIN))
    CO = np.cos(th) * win[None, :] / M                   # [1024,512]
    SO = np.sin(th) * win[None, :] / M

    bigDs, zcs = [], []
    for c in range(NCORES):
        sl = slice(c * FS, (c + 1) * FS)
        bigD = np.concatenate([CO[sl, :], SO[sl, :]], axis=1).astype(np.float16)
        zcp = ZC[:, sl].reshape(4, 128, FS).transpose(1, 0, 2).reshape(128, 512)
        zsp = ZS[:, sl].reshape(4, 128, FS).transpose(1, 0, 2).reshape(128, 512)
        zcs.append(np.concatenate([zcp, zsp], axis=1).astype(np.float16))
        bigDs.append(np.ascontiguousarray(bigD))
    return CF.astype(np.float32), SF.astype(np.float32), bigDs, zcs


_CF, _SF, _BIGD, _ZCS = _make_constants()
_NC = None


# ---------------- device program ----------------
def _build_nc():
    nc = bacc.Bacc()
    a_e = nc.dram_tensor("bigA", [128, CA], F16, kind="ExternalInput")
    h_e = nc.dram_tensor("bigH", [128, CH], F16, kind="ExternalInput")
    d_e = nc.dram_tensor("bigD", [128, CD], F16, kind="ExternalInput")
    e_e = nc.dram_tensor("bigE", [128, CE], F16, kind="ExternalInput")
    out_e = nc.dram_tensor("out", [B, T * HOP], F16, kind="ExternalOutput")

    with tile.TileContext(nc) as tc:
        with tc.tile_pool(name="sb", bufs=1) as sb, \
             tc.tile_pool(name="ps", bufs=1, space="PSUM") as ps:

            # ---- input DMAs: hardware dynamic queues only ----
            bigA = sb.tile([128, CA], F16, tag="bigA", name="bigA")
            nc.sync.dma_start(out=bigA[:], in_=a_e[:, :])
            dummy = sb.tile([128, 512], F16, tag="dummy", name="dummy")
            nc.gpsimd.memset(dummy[:, :], 0.0)
            bigE = sb.tile([128, CE], F16, tag="bigE", name="bigE")
            nc.scalar.dma_start(out=bigE[:], in_=e_e[:, :])
            bigH = sb.tile([128, CH], F16, tag="bigH", name="bigH")
            nc.sync.dma_start(out=bigH[:], in_=h_e[:, :])
            bigD = sb.tile([128, CD], F16, tag="bigD", name="bigD")
            nc.sync.dma_start(out=bigD[:], in_=d_e[:, :])

            xcatA = bigA[0:121, 0:256]
            xcatB = bigA[0:120, 256:512]
            GA = bigA[0:121, 512:640]
            GB = bigA[0:120, 640:768]
            HA = bigH[0:121, 0:128]
            HB = bigH[0:120, 128:256]
            fr = bigE[:, 0:1024]
            zc = bigE[:, 1024:1536]
            zs = bigE[:, 1536:2048]
            co = bigD[:, 0:512]
            so = bigD[:, 512:1024]

            # ---- PSUM tiles (separate tags -> minimal tile-granular waits)
            trash = ps.tile([128, 256], F32, tag="ob0", name="trash")
            yrp = ps.tile([128, 256], F32, tag="yr", name="yrp")
            yip = ps.tile([128, 256], F32, tag="yi", name="yip")
            zrp = ps.tile([128, 256], F32, tag="zr", name="zrp")
            zip_ = ps.tile([128, 256], F32, tag="zi", name="zip")

            def warm(n):
                for _ in range(n):
                    nc.tensor.matmul(trash[:, :], dummy[:, 0:128],
                                     dummy[:, 0:256], start=True, stop=True)

            # ---- PE warmup: bridge the bigA DMA wait, keep HAM busy ----
            warm(6)

            # ---- dummy Silu pins the ACT table set (one load, at t~0) ----
            scr = sb.tile([128, 8], F32, tag="scr", name="scr")
            nc.scalar.activation(scr[:, :], dummy[:, 0:8], AF.Silu)

            # ---- Yr/Yi [f_local, bt] straight from xcat ----
            nc.tensor.matmul(yrp[:, :], GA, xcatA, start=True, stop=False)
            nc.tensor.matmul(yrp[:, :], GB, xcatB, start=False, stop=True)
            nc.tensor.matmul(yip[:, :], HA, xcatA, start=True, stop=False)
            nc.tensor.matmul(yip[:, :], HB, xcatB, start=False, stop=True)
            warm(2)

            # ---- Zr/Zi [f_local, bt] (overlaps the act chain) ----
            for mc in range(4):
                nc.tensor.matmul(zrp[:, :], zc[:, mc * FS:(mc + 1) * FS],
                                 fr[:, mc * BT:(mc + 1) * BT],
                                 start=(mc == 0), stop=(mc == 3))
            for mc in range(4):
                nc.tensor.matmul(zip_[:, :], zs[:, mc * FS:(mc + 1) * FS],
                                 fr[:, mc * BT:(mc + 1) * BT],
                                 start=(mc == 0), stop=(mc == 3))
            warm(2)

            # ---- act chain: mag=(1+t)/(1-t), sin, cos ----
            def wt(name, dt=F32, w_=256):
                return sb.tile([128, w_], dt, tag=name, name=name)

            th = wt("th")
            nc.scalar.activation(th[:, :], yrp[:, :], AF.Tanh)
            yiw = wt("yiw")
            nc.vector.add_range_wrap(yiw[:, :], yip[:, :], 0.0, PI, 2.0 * PI)
            yic = wt("yic")
            nc.vector.add_range_wrap(yic[:, :], yip[:, :], PI / 2.0, PI,
                                     2.0 * PI)
            sinv = wt("sinv")
            nc.scalar.activation(sinv[:, :], yiw[:, :], AF.Sin)
            cosv = wt("cosv")
            nc.scalar.activation(cosv[:, :], yic[:, :], AF.Sin)
            zis = wt("zis", F16)
            nc.scalar.copy(zis[:, :], zip_[:, :])
            zrs = wt("zrs", F16)
            nc.scalar.copy(zrs[:, :], zrp[:, :])
            den = wt("den")
            nc.gpsimd.tensor_scalar(den[:, :], th[:, :], -1.0, 1.0,
                                    ALU.mult, ALU.add)
            rf = wt("rf")
            nc.vector.reciprocal_approx_fast(rf[:, :], den[:, :])
            magv = wt("magv")
            nc.vector.scalar_tensor_tensor(magv[:, :], th[:, :], 1.0, rf[:, :],
                                           ALU.add, ALU.mult)
            Av = wt("Av")
            nc.vector.tensor_tensor(Av[:, :], magv[:, :], cosv[:, :], ALU.mult)
            Bv = wt("Bv", F16)
            nc.vector.tensor_tensor(Bv[:, :], magv[:, :], sinv[:, :], ALU.mult)

            # ---- P = (Av + iBv)(Zr + iZi), fp16 padded stationary layout ----
            t1 = wt("t1", F16)
            nc.vector.tensor_tensor(t1[:, :], Av[:, :], zrp[:, :], ALU.mult)
            t3 = wt("t3", F16)
            nc.vector.tensor_tensor(t3[:, :], Av[:, :], zip_[:, :], ALU.mult)
            t2 = wt("t2", F16)
            nc.vector.tensor_tensor(t2[:, :], Bv[:, :], zis[:, :], ALU.mult)
            t4 = wt("t4", F16)
            nc.vector.tensor_tensor(t4[:, :], Bv[:, :], zrs[:, :], ALU.mult)
            PrP = wt("PrP", F16, 260)   # per batch: [wrap, t0..t127] at b*130
            PiP = wt("PiP", F16, 260)
            for b in range(B):
                sl = slice(b * T, (b + 1) * T)
                nc.vector.tensor_tensor(PrP[:, b * 130 + 1:b * 130 + 129],
                                        t1[:, sl], t2[:, sl], ALU.subtract)
                nc.vector.tensor_tensor(PiP[:, b * 130 + 1:b * 130 + 129],
                                        t3[:, sl], t4[:, sl], ALU.add)
            for b in range(B):
                nc.vector.tensor_copy(PrP[:, b * 130:b * 130 + 1],
                                      PrP[:, b * 130 + 128:b * 130 + 129])
                nc.gpsimd.tensor_copy(PiP[:, b * 130:b * 130 + 1],
                                      PiP[:, b * 130 + 128:b * 130 + 129])

            # ---- step6 with OLA folded in ----
            obs = []
            for b in range(B):
                ob = trash if b == 0 else ps.tile([128, 256], F32, tag="ob1",
                                                  name="ob1")
                u = b * 130 + 1   # unshifted stationary cols; u-1 = shifted
                nc.tensor.matmul(ob[:, :], PrP[:, u:u + 128], co[:, 0:256],
                                 start=True, stop=False)
                nc.tensor.matmul(ob[:, :], PiP[:, u:u + 128], so[:, 0:256],
                                 start=False, stop=False)
                nc.tensor.matmul(ob[:, :], PrP[:, u - 1:u + 127],
                                 co[:, 256:512], start=False, stop=False)
                nc.tensor.matmul(ob[:, :], PiP[:, u - 1:u + 127],
                                 so[:, 256:512], start=False, stop=True)
                ot = sb.tile([128, 256], F16, tag=f"obs{b}", name=f"obs{b}")
                nc.scalar.copy(ot[:, :], ob[:, :])
                obs.append(ot)

            for b in range(B):
                dst = bass.AP(out_e[:, :].tensor, b * T * HOP,
                              [[HOP, T], [1, HOP]])
                eng = nc.sync if b == 0 else nc.scalar
                eng.dma_start(out=dst, in_=obs[b][:, :])

    return nc


def _get_nc():
    global _NC
    if _NC is None:
        _NC = _build_nc()
        _NC.finalize()
    return _NC


# ---------------- host orchestration ----------------
def kernel(x, z, W, b):
    global LAST_RESULT
    x = np.ascontiguousarray(np.asarray(x, dtype=np.float32))
    z = np.ascontiguousarray(np.asarray(z, dtype=np.float32))
    W = np.ascontiguousarray(np.asarray(W, dtype=np.float32))
    b = np.ascontiguousarray(np.asarray(b, dtype=np.float32))

    # xcat: 3 shifted copies of x^T + ones row -> [241, 256]
    xT = np.ascontiguousarray(x.reshape(BT, D).T)                 # [80, 256]
    xsh = np.zeros((3, D, BT), np.float32)
    xsh[1] = xT
    xv = xT.reshape(D, B, T)
    xsh[0].reshape(D, B, T)[:, :, 1:] = xv[:, :, :-1]
    xsh[2].reshape(D, B, T)[:, :, :-1] = xv[:, :, 1:]
    xcat = np.concatenate([xsh.reshape(3 * D, BT),
                           np.ones((1, BT), np.float32)], axis=0)  # [241,256]
    w2 = np.concatenate([W[:, :, 0].T, W[:, :, 1].T, W[:, :, 2].T,
                         b[None, :]], axis=0)                      # [241,222]
    G = (w2 @ _CF).astype(np.float16)                              # [241,1024]
    H = (w2 @ _SF).astype(np.float16)

    # frames^T: fr[u_low, mc*BT + b*T + t] = zpad[b, t*HOP + mc*128 + u_low]
    zpad = np.concatenate(
        [np.zeros((B, HOP), np.float32), z[:, 0, :]], axis=1)     # [2, 33024]
    fidx = (np.arange(T)[:, None] * HOP + np.arange(WIN)[None, :])
    frames = zpad[:, fidx]                                        # [B,T,WIN]
    fr = frames.reshape(B, T, 4, 128).transpose(3, 2, 0, 1) \
        .reshape(128, 4 * BT).astype(np.float16)

    xc16 = xcat.astype(np.float16)
    in_maps = []
    for c in range(NCORES):
        sl = slice(c * FS, (c + 1) * FS)
        a = np.zeros((128, CA), np.float16)
        a[0:121, 0:256] = xc16[0:121]
        a[0:120, 256:512] = xc16[121:241]
        a[0:121, 512:640] = G[0:121, sl]
        a[0:120, 640:768] = G[121:241, sl]
        h = np.zeros((128, CH), np.float16)
        h[0:121, 0:128] = H[0:121, sl]
        h[0:120, 128:256] = H[121:241, sl]
        e = np.concatenate([fr, _ZCS[c]], axis=1)
        in_maps.append({"bigA": a, "bigH": h, "bigD": _BIGD[c],
                        "bigE": np.ascontiguousarray(e)})

    nc = _get_nc()
    res = run_bass_kernel_spmd(nc, in_maps, list(range(NCORES)), trace=TRACE)
    LAST_RESULT = res
    out = np.zeros((B, T * HOP), dtype=np.float32)
    for r in res.results:
        out += np.asarray(r["out"], dtype=np.float32)
    return out.reshape(B, 1, T * HOP)


# revision 16
# speedup vs baseline: 1.0335x; 1.0335x over previous
"""Trainium2 Bass kernel for nn_CCepLTVFilter (v5).

Frequency-sharded across 8 cores (128 freqs each); every core computes all
256 (b,t) frames; per-core outputs are partial sums of the full output.

Device pipeline:
  1. Yr/Yi = G^T @ xcat, H^T @ xcat   (PE; G = w2@CF, H = w2@SF folded on
     host, so the conv stage disappears from the device critical path)
  2. mag=(1+t)/(1-t), t=tanh(Yr); sin/cos via Sin ACT — single table set
     (pinned by a dummy Silu so no mid-chain ACT table switch)
  3. Zr/Zi = DFT of frames (PE; frames pre-transposed on host)
  4. P = (mag cos + i mag sin)(Zr + iZi)   (DVE; fp16 tail products)
  5. step6+OLA fused: ob[t,h] = P@CO_l/SO_l + P(t-1)@CO_r/SO_r with the
     frame roll expressed as a shifted stationary slice (wrap column).
All matmul operands fp16, fp32 PSUM accumulation; fp16 partial outputs
summed in fp32 on host. DMAs only on sync/scalar hardware queues.
Separate PSUM tiles per result so tile-granular waits stay minimal.
PE warmup dummies bridge DMA waits and keep the HAM clock-gate open.
"""

import numpy as np

import concourse.bass as bass
import concourse.bacc as bacc
import concourse.mybir as mybir
import concourse.tile as tile
from concourse.bass_utils import run_bass_kernel_spmd

# ---------------- problem dims (hardcoded) ----------------
B, T, D = 2, 128, 80
CCEP = 222
FFT = 1024
HOP = 256
WIN = 2 * HOP            # 512
PAD = (FFT - CCEP) // 2  # 401
M = FFT + 1              # 1025-point transforms
BT = B * T               # 256
NCORES = 8
FS = FFT // NCORES       # 128 frequencies per core
LAM = float(np.log(10.0) / 10.0)

F16 = mybir.dt.float16
F32 = mybir.dt.float32
PI = float(np.pi)
AF = mybir.ActivationFunctionType
ALU = mybir.AluOpType

# bigA cols (fp16): xcatA[0:256) xcatB[256:512) GA[512:640) GB[640:768)
#   HA[768:896) HB[896:1024)
CA = 1024
CD = 1024                # bigD: co[0:512) so[512:1024)
CE = 2048                # bigE: fr[0:1024) zc[1024:1536) zs[1536:2048)

TRACE = False
LAST_RESULT = None


# ---------------- host-side constants (input independent) ----------------
def _make_constants():
    o = np.arange(CCEP, dtype=np.float64)[:, None]
    f = np.arange(FFT, dtype=np.float64)[None, :]
    qn_idx = np.arange(1, CCEP // 2 + 1, dtype=np.float64)
    qnorm = np.concatenate([qn_idx[::-1], qn_idx])
    ang = 2.0 * np.pi * f * (o + PAD) / FFT
    CF = np.cos(ang) * (LAM / 2.0) / qnorm[:, None]      # [222,1024]
    SF = -np.sin(ang) / qnorm[:, None]

    u = np.arange(WIN, dtype=np.float64)[:, None]
    phi = 2.0 * np.pi * f * (u + FFT // 2) / M
    ZC = np.cos(phi)                                     # [512,1024]
    ZS = np.sin(phi)

    w = np.arange(WIN, dtype=np.float64)[None, :]
    th = 2.0 * np.pi * np.arange(FFT, dtype=np.float64)[:, None] * w / M
    win = 0.5 * (1.0 - np.cos(2.0 * np.pi * np.arange(WIN) / WIN))
    CO = np.cos(th) * win[None, :] / M                   # [1024,512]
    SO = np.sin(th) * win[None, :] / M

    bigDs, zcs = [], []
    for c in range(NCORES):
        sl = slice(c * FS, (c + 1) * FS)
        bigD = np.concatenate([CO[sl, :], SO[sl, :]], axis=1).astype(np.float16)
        zcp = ZC[:, sl].reshape(4, 128, FS).transpose(1, 0, 2).reshape(128, 512)
        zsp = ZS[:, sl].reshape(4, 128, FS).transpose(1, 0, 2).reshape(128, 512)
        zcs.append(np.concatenate([zcp, zsp], axis=1).astype(np.float16))
        bigDs.append(np.ascontiguousarray(bigD))
    return CF.astype(np.float32), SF.astype(np.float32), bigDs, zcs


_CF, _SF, _BIGD, _ZCS = _make_constants()
_NC = None


# ---------------- device program ----------------
def _build_nc():
    nc = bacc.Bacc()
    a_e = nc.dram_tensor("bigA", [128, CA], F16, kind="ExternalInput")
    d_e = nc.dram_tensor("bigD", [128, CD], F16, kind="ExternalInput")
    e_e = nc.dram_tensor("bigE", [128, CE], F16, kind="ExternalInput")
    out_e = nc.dram_tensor("out", [B, T * HOP], F16, kind="ExternalOutput")

    with tile.TileContext(nc) as tc:
        with tc.tile_pool(name="sb", bufs=1) as sb, \
             tc.tile_pool(name="ps", bufs=1, space="PSUM") as ps:

            # ---- input DMAs: hardware dynamic queues only ----
            bigA = sb.tile([128, CA], F16, tag="bigA", name="bigA")
            nc.sync.dma_start(out=bigA[:], in_=a_e[:, :])
            dummy = sb.tile([128, 512], F16, tag="dummy", name="dummy")
            nc.gpsimd.memset(dummy[:, :], 0.0)
            bigE = sb.tile([128, CE], F16, tag="bigE", name="bigE")
            nc.scalar.dma_start(out=bigE[:], in_=e_e[:, :])
            bigD = sb.tile([128, CD], F16, tag="bigD", name="bigD")
            nc.sync.dma_start(out=bigD[:], in_=d_e[:, :])

            xcatA = bigA[0:121, 0:256]
            xcatB = bigA[0:120, 256:512]
            GA = bigA[0:121, 512:640]
            GB = bigA[0:120, 640:768]
            HA = bigA[0:121, 768:896]
            HB = bigA[0:120, 896:1024]
            fr = bigE[:, 0:1024]
            zc = bigE[:, 1024:1536]
            zs = bigE[:, 1536:2048]
            co = bigD[:, 0:512]
            so = bigD[:, 512:1024]

            # ---- PSUM tiles (separate tags -> minimal tile-granular waits)
            trash = ps.tile([128, 256], F32, tag="ob0", name="trash")
            yrp = ps.tile([128, 256], F32, tag="yr", name="yrp")
            yip = ps.tile([128, 256], F32, tag="yi", name="yip")
            zrp = ps.tile([128, 256], F32, tag="zr", name="zrp")
            zip_ = ps.tile([128, 256], F32, tag="zi", name="zip")

            def warm(n):
                for _ in range(n):
                    nc.tensor.matmul(trash[:, :], dummy[:, 0:128],
                                     dummy[:, 0:256], start=True, stop=True)

            # ---- PE warmup: bridge the bigA DMA wait, keep HAM busy ----
            warm(6)

            # ---- dummy Silu pins the ACT table set (one load, at t~0) ----
            scr = sb.tile([128, 8], F32, tag="scr", name="scr")
            nc.scalar.activation(scr[:, :], dummy[:, 0:8], AF.Silu)

            # ---- Yr/Yi [f_local, bt] straight from xcat ----
            nc.tensor.matmul(yrp[:, :], GA, xcatA, start=True, stop=False)
            nc.tensor.matmul(yrp[:, :], GB, xcatB, start=False, stop=True)
            nc.tensor.matmul(yip[:, :], HA, xcatA, start=True, stop=False)
            nc.tensor.matmul(yip[:, :], HB, xcatB, start=False, stop=True)

            # ---- Zr/Zi [f_local, bt] (overlaps the act chain) ----
            for mc in range(4):
                nc.tensor.matmul(zrp[:, :], zc[:, mc * FS:(mc + 1) * FS],
                                 fr[:, mc * BT:(mc + 1) * BT],
                                 start=(mc == 0), stop=(mc == 3))
            for mc in range(4):
                nc.tensor.matmul(zip_[:, :], zs[:, mc * FS:(mc + 1) * FS],
                                 fr[:, mc * BT:(mc + 1) * BT],
                                 start=(mc == 0), stop=(mc == 3))

            # ---- act chain: mag=(1+t)/(1-t), sin, cos ----
            def wt(name, dt=F32, w_=256):
                return sb.tile([128, w_], dt, tag=name, name=name)

            th = wt("th")
            nc.scalar.activation(th[:, :], yrp[:, :], AF.Tanh)
            yiw = wt("yiw")
            nc.vector.add_range_wrap(yiw[:, :], yip[:, :], 0.0, PI, 2.0 * PI)
            yic = wt("yic")
            nc.vector.add_range_wrap(yic[:, :], yip[:, :], PI / 2.0, PI,
                                     2.0 * PI)
            sinv = wt("sinv")
            nc.scalar.activation(sinv[:, :], yiw[:, :], AF.Sin)
            cosv = wt("cosv")
            nc.scalar.activation(cosv[:, :], yic[:, :], AF.Sin)
            zis = wt("zis", F16)
            nc.scalar.copy(zis[:, :], zip_[:, :])
            zrs = wt("zrs", F16)
            nc.scalar.copy(zrs[:, :], zrp[:, :])
            den = wt("den")
            nc.gpsimd.tensor_scalar(den[:, :], th[:, :], -1.0, 1.0,
                                    ALU.mult, ALU.add)
            rf = wt("rf")
            nc.vector.reciprocal_approx_fast(rf[:, :], den[:, :])
            magv = wt("magv")
            nc.vector.scalar_tensor_tensor(magv[:, :], th[:, :], 1.0, rf[:, :],
                                           ALU.add, ALU.mult)
            Av = wt("Av")
            nc.vector.tensor_tensor(Av[:, :], magv[:, :], cosv[:, :], ALU.mult)
            Bv = wt("Bv", F16)
            nc.vector.tensor_tensor(Bv[:, :], magv[:, :], sinv[:, :], ALU.mult)

            # ---- P = (Av + iBv)(Zr + iZi), fp16 padded stationary layout ----
            t1 = wt("t1", F16)
            nc.vector.tensor_tensor(t1[:, :], Av[:, :], zrp[:, :], ALU.mult)
            t3 = wt("t3", F16)
            nc.vector.tensor_tensor(t3[:, :], Av[:, :], zip_[:, :], ALU.mult)
            t2 = wt("t2", F16)
            nc.vector.tensor_tensor(t2[:, :], Bv[:, :], zis[:, :], ALU.mult)
            t4 = wt("t4", F16)
            nc.vector.tensor_tensor(t4[:, :], Bv[:, :], zrs[:, :], ALU.mult)
            PrP = wt("PrP", F16, 260)   # per batch: [wrap, t0..t127] at b*130
            PiP = wt("PiP", F16, 260)
            for b in range(B):
                sl = slice(b * T, (b + 1) * T)
                nc.vector.tensor_tensor(PrP[:, b * 130 + 1:b * 130 + 129],
                                        t1[:, sl], t2[:, sl], ALU.subtract)
                nc.vector.tensor_tensor(PiP[:, b * 130 + 1:b * 130 + 129],
                                        t3[:, sl], t4[:, sl], ALU.add)
            for b in range(B):
                nc.vector.tensor_copy(PrP[:, b * 130:b * 130 + 1],
                                      PrP[:, b * 130 + 128:b * 130 + 129])
                nc.gpsimd.tensor_copy(PiP[:, b * 130:b * 130 + 1],
                                      PiP[:, b * 130 + 128:b * 130 + 129])

            # ---- step6 with OLA folded in ----
            obs = []
            for b in range(B):
                ob = trash if b == 0 else ps.tile([128, 256], F32, tag="ob1",
                                                  name="ob1")
                u = b * 130 + 1   # unshifted stationary cols; u-1 = shifted
                nc.tensor.matmul(ob[:, :], PrP[:, u:u + 128], co[:, 0:256],
                                 start=True, stop=False)
                nc.tensor.matmul(ob[:, :], PiP[:, u:u + 128], so[:, 0:256],
                                 start=False, stop=False)
                nc.tensor.matmul(ob[:, :], PrP[:, u - 1:u + 127],
                                 co[:, 256:512], start=False, stop=False)
                nc.tensor.matmul(ob[:, :], PiP[:, u - 1:u + 127],
                                 so[:, 256:512], start=False, stop=True)
                ot = sb.tile([128, 256], F16, tag=f"obs{b}", name=f"obs{b}")
                nc.scalar.copy(ot[:, :], ob[:, :])
                obs.append(ot)

            for b in range(B):
                dst = bass.AP(out_e[:, :].tensor, b * T * HOP,
                              [[HOP, T], [1, HOP]])
                eng = nc.sync if b == 0 else nc.scalar
                eng.dma_start(out=dst, in_=obs[b][:, :])

    return nc


def _get_nc():
    global _NC
    if _NC is None:
        _NC = _build_nc()
        _NC.finalize()
    return _NC


# ---------------- host orchestration ----------------
def kernel(x, z, W, b):
    global LAST_RESULT
    x = np.ascontiguousarray(np.asarray(x, dtype=np.float32))
    z = np.ascontiguousarray(np.asarray(z, dtype=np.float32))
    W = np.ascontiguousarray(np.asarray(W, dtype=np.float32))
    b = np.ascontiguousarray(np.asarray(b, dtype=np.float32))

    # xcat: 3 shifted copies of x^T + ones row -> [241, 256]
    xT = np.ascontiguousarray(x.reshape(BT, D).T)                 # [80, 256]
    xsh = np.zeros((3, D, BT), np.float32)
    xsh[1] = xT
    xv = xT.reshape(D, B, T)
    xsh[0].reshape(D, B, T)[:, :, 1:] = xv[:, :, :-1]
    xsh[2].reshape(D, B, T)[:, :, :-1] = xv[:, :, 1:]
    xcat = np.concatenate([xsh.reshape(3 * D, BT),
                           np.ones((1, BT), np.float32)], axis=0)  # [241,256]
    w2 = np.concatenate([W[:, :, 0].T, W[:, :, 1].T, W[:, :, 2].T,
                         b[None, :]], axis=0)                      # [241,222]
    G = (w2 @ _CF).astype(np.float16)                              # [241,1024]
    H = (w2 @ _SF).astype(np.float16)

    # frames^T: fr[u_low, mc*BT + b*T + t] = zpad[b, t*HOP + mc*128 + u_low]
    zpad = np.concatenate(
        [np.zeros((B, HOP), np.float32), z[:, 0, :]], axis=1)     # [2, 33024]
    fidx = (np.arange(T)[:, None] * HOP + np.arange(WIN)[None, :])
    frames = zpad[:, fidx]                                        # [B,T,WIN]
    fr = frames.reshape(B, T, 4, 128).transpose(3, 2, 0, 1) \
        .reshape(128, 4 * BT).astype(np.float16)

    xc16 = xcat.astype(np.float16)
    in_maps = []
    for c in range(NCORES):
        sl = slice(c * FS, (c + 1) * FS)
        a = np.zeros((128, CA), np.float16)
        a[0:121, 0:256] = xc16[0:121]
        a[0:120, 256:512] = xc16[121:241]
        a[0:121, 512:640] = G[0:121, sl]
        a[0:120, 640:768] = G[121:241, sl]
        a[0:121, 768:896] = H[0:121, sl]
        a[0:120, 896:1024] = H[121:241, sl]
        e = np.concatenate([fr, _ZCS[c]], axis=1)
        in_maps.append({"bigA": a, "bigD": _BIGD[c],
                        "bigE": np.ascontiguousarray(e)})

    nc = _get_nc()
    res = run_bass_kernel_spmd(nc, in_maps, list(range(NCORES)), trace=TRACE)
    LAST_RESULT = res
    out = np.zeros((B, T * HOP), dtype=np.float32)
    for r in res.results:
        out += np.asarray(r["out"], dtype=np.float32)
    return out.reshape(B, 1, T * HOP)


# revision 19
# speedup vs baseline: 1.0366x; 1.0030x over previous
"""Trainium2 Bass kernel for nn_CCepLTVFilter (v5).

Frequency-sharded across 8 cores (128 freqs each); every core computes all
256 (b,t) frames; per-core outputs are partial sums of the full output.

Device pipeline:
  1. Yr/Yi = G^T @ xcat, H^T @ xcat   (PE; G = w2@CF, H = w2@SF folded on
     host, so the conv stage disappears from the device critical path)
  2. mag=(1+t)/(1-t), t=tanh(Yr); sin/cos via Sin ACT — single table set
     (pinned by a dummy Silu so no mid-chain ACT table switch)
  3. Zr/Zi = DFT of frames (PE; frames pre-transposed on host)
  4. P = (mag cos + i mag sin)(Zr + iZi)   (DVE; fp16 tail products)
  5. step6+OLA fused: ob[t,h] = P@CO_l/SO_l + P(t-1)@CO_r/SO_r with the
     frame roll expressed as a shifted stationary slice (wrap column).
All matmul operands fp16, fp32 PSUM accumulation; fp16 partial outputs
summed in fp32 on host. DMAs only on sync/scalar hardware queues.
Separate PSUM tiles per result so tile-granular waits stay minimal.
PE warmup dummies bridge DMA waits and keep the HAM clock-gate open.
"""

import numpy as np

import concourse.bass as bass
import concourse.bacc as bacc
import concourse.mybir as mybir
import concourse.tile as tile
from concourse.bass_utils import run_bass_kernel_spmd

# ---------------- problem dims (hardcoded) ----------------
B, T, D = 2, 128, 80
CCEP = 222
FFT = 1024
HOP = 256
WIN = 2 * HOP            # 512
PAD = (FFT - CCEP) // 2  # 401
M = FFT + 1              # 1025-point transforms
BT = B * T               # 256
NCORES = 8
FS = FFT // NCORES       # 128 frequencies per core
LAM = float(np.log(10.0) / 10.0)

F16 = mybir.dt.float16
F32 = mybir.dt.float32
PI = float(np.pi)
AF = mybir.ActivationFunctionType
ALU = mybir.AluOpType

# bigA cols (fp16): xcatA[0:256) xcatB[256:512) GA[512:640) GB[640:768)
#   HA[768:896) HB[896:1024)
CA = 1024
CD = 1024                # bigD: co[0:512) so[512:1024)
CE = 2048                # bigE: fr[0:1024) zc[1024:1536) zs[1536:2048)

TRACE = False
LAST_RESULT = None


# ---------------- host-side constants (input independent) ----------------
def _make_constants():
    o = np.arange(CCEP, dtype=np.float64)[:, None]
    f = np.arange(FFT, dtype=np.float64)[None, :]
    qn_idx = np.arange(1, CCEP // 2 + 1, dtype=np.float64)
    qnorm = np.concatenate([qn_idx[::-1], qn_idx])
    ang = 2.0 * np.pi * f * (o + PAD) / FFT
    CF = np.cos(ang) * (LAM / 2.0) / qnorm[:, None]      # [222,1024]
    SF = -np.sin(ang) / qnorm[:, None]

    u = np.arange(WIN, dtype=np.float64)[:, None]
    phi = 2.0 * np.pi * f * (u + FFT // 2) / M
    ZC = np.cos(phi)                                     # [512,1024]
    ZS = np.sin(phi)

    w = np.arange(WIN, dtype=np.float64)[None, :]
    th = 2.0 * np.pi * np.arange(FFT, dtype=np.float64)[:, None] * w / M
    win = 0.5 * (1.0 - np.cos(2.0 * np.pi * np.arange(WIN) / WIN))
    CO = np.cos(th) * win[None, :] / M                   # [1024,512]
    SO = np.sin(th) * win[None, :] / M

    bigDs, zcs = [], []
    for c in range(NCORES):
        sl = slice(c * FS, (c + 1) * FS)
        bigD = np.concatenate([CO[sl, :], SO[sl, :]], axis=1).astype(np.float16)
        zcp = ZC[:, sl].reshape(4, 128, FS).transpose(1, 0, 2).reshape(128, 512)
        zsp = ZS[:, sl].reshape(4, 128, FS).transpose(1, 0, 2).reshape(128, 512)
        zcs.append(np.concatenate([zcp, zsp], axis=1).astype(np.float16))
        bigDs.append(np.ascontiguousarray(bigD))
    return CF.astype(np.float32), SF.astype(np.float32), bigDs, zcs


_CF, _SF, _BIGD, _ZCS = _make_constants()
_NC = None


# ---------------- device program ----------------
def _build_nc():
    nc = bacc.Bacc()
    a_e = nc.dram_tensor("bigA", [128, CA], F16, kind="ExternalInput")
    d_e = nc.dram_tensor("bigD", [128, CD], F16, kind="ExternalInput")
    e_e = nc.dram_tensor("bigE", [128, CE], F16, kind="ExternalInput")
    out_e = nc.dram_tensor("out", [B, T * HOP], F16, kind="ExternalOutput")

    with tile.TileContext(nc) as tc:
        with tc.tile_pool(name="sb", bufs=1) as sb, \
             tc.tile_pool(name="ps", bufs=1, space="PSUM") as ps:

            # ---- input DMAs: hardware dynamic queues only ----
            bigA = sb.tile([128, CA], F16, tag="bigA", name="bigA")
            nc.sync.dma_start(out=bigA[:], in_=a_e[:, :])
            dummy = sb.tile([128, 512], F16, tag="dummy", name="dummy")
            nc.gpsimd.memset(dummy[:, :], 0.0)
            bigE = sb.tile([128, CE], F16, tag="bigE", name="bigE")
            nc.scalar.dma_start(out=bigE[:], in_=e_e[:, :])
            bigD = sb.tile([128, CD], F16, tag="bigD", name="bigD")
            nc.sync.dma_start(out=bigD[:], in_=d_e[:, :])

            xcatA = bigA[0:121, 0:256]
            xcatB = bigA[0:120, 256:512]
            GA = bigA[0:121, 512:640]
            GB = bigA[0:120, 640:768]
            HA = bigA[0:121, 768:896]
            HB = bigA[0:120, 896:1024]
            fr = bigE[:, 0:1024]
            zc = bigE[:, 1024:1536]
            zs = bigE[:, 1536:2048]
            co = bigD[:, 0:512]
            so = bigD[:, 512:1024]

            # ---- PSUM tiles (separate tags -> minimal tile-granular waits)
            trash = ps.tile([128, 256], F32, tag="ob0", name="trash")
            yrp = ps.tile([128, 256], F32, tag="yr", name="yrp")
            yip = ps.tile([128, 256], F32, tag="yi", name="yip")
            zrp = ps.tile([128, 256], F32, tag="zr", name="zrp")
            zip_ = ps.tile([128, 256], F32, tag="zi", name="zip")

            def warm(n):
                for _ in range(n):
                    nc.tensor.matmul(trash[:, :], dummy[:, 0:128],
                                     dummy[:, 0:256], start=True, stop=True)

            # ---- PE warmup: bridge the bigA DMA wait, keep HAM busy ----
            warm(6)

            # ---- dummy Silu pins the ACT table set (one load, at t~0) ----
            scr = sb.tile([128, 8], F32, tag="scr", name="scr")
            nc.scalar.activation(scr[:, :], dummy[:, 0:8], AF.Silu)

            # ---- Yr/Yi [f_local, bt] straight from xcat ----
            nc.tensor.matmul(yrp[:, :], GA, xcatA, start=True, stop=False)
            nc.tensor.matmul(yrp[:, :], GB, xcatB, start=False, stop=True)
            nc.tensor.matmul(yip[:, :], HA, xcatA, start=True, stop=False)
            nc.tensor.matmul(yip[:, :], HB, xcatB, start=False, stop=True)

            # ---- Zr/Zi [f_local, bt] (overlaps the act chain) ----
            for mc in range(4):
                nc.tensor.matmul(zrp[:, :], zc[:, mc * FS:(mc + 1) * FS],
                                 fr[:, mc * BT:(mc + 1) * BT],
                                 start=(mc == 0), stop=(mc == 3))
            for mc in range(4):
                nc.tensor.matmul(zip_[:, :], zs[:, mc * FS:(mc + 1) * FS],
                                 fr[:, mc * BT:(mc + 1) * BT],
                                 start=(mc == 0), stop=(mc == 3))

            # ---- act chain: mag=(1+t)/(1-t), sin, cos ----
            def wt(name, dt=F32, w_=256):
                return sb.tile([128, w_], dt, tag=name, name=name)

            th = wt("th")
            nc.scalar.activation(th[:, :], yrp[:, :], AF.Tanh)
            yiw = wt("yiw")
            nc.vector.add_range_wrap(yiw[:, :], yip[:, :], 0.0, PI, 2.0 * PI)
            yic = wt("yic")
            nc.vector.add_range_wrap(yic[:, :], yip[:, :], PI / 2.0, PI,
                                     2.0 * PI)
            sinv = wt("sinv")
            nc.scalar.activation(sinv[:, :], yiw[:, :], AF.Sin)
            cosv = wt("cosv")
            nc.scalar.activation(cosv[:, :], yic[:, :], AF.Sin)
            den = wt("den")
            nc.gpsimd.tensor_scalar(den[:, :], th[:, :], -1.0, 1.0,
                                    ALU.mult, ALU.add)
            rf = wt("rf")
            nc.vector.reciprocal_approx_fast(rf[:, :], den[:, :])
            magv = wt("magv")
            nc.vector.scalar_tensor_tensor(magv[:, :], th[:, :], 1.0, rf[:, :],
                                           ALU.add, ALU.mult)
            Av = wt("Av")
            nc.vector.tensor_tensor(Av[:, :], magv[:, :], cosv[:, :], ALU.mult)
            Bv = wt("Bv")
            nc.vector.tensor_tensor(Bv[:, :], magv[:, :], sinv[:, :], ALU.mult)

            # ---- P = (Av + iBv)(Zr + iZi), fp16 padded stationary layout ----
            t1 = wt("t1", F16)
            nc.vector.tensor_tensor(t1[:, :], Av[:, :], zrp[:, :], ALU.mult)
            t3 = wt("t3", F16)
            nc.vector.tensor_tensor(t3[:, :], Av[:, :], zip_[:, :], ALU.mult)
            t2 = wt("t2", F16)
            nc.vector.tensor_tensor(t2[:, :], Bv[:, :], zip_[:, :], ALU.mult)
            t4 = wt("t4", F16)
            nc.vector.tensor_tensor(t4[:, :], Bv[:, :], zrp[:, :], ALU.mult)
            PrP = wt("PrP", F16, 260)   # per batch: [wrap, t0..t127] at b*130
            PiP = wt("PiP", F16, 260)
            for b in range(B):
                sl = slice(b * T, (b + 1) * T)
                nc.vector.tensor_tensor(PrP[:, b * 130 + 1:b * 130 + 129],
                                        t1[:, sl], t2[:, sl], ALU.subtract)
                nc.vector.tensor_tensor(PiP[:, b * 130 + 1:b * 130 + 129],
                                        t3[:, sl], t4[:, sl], ALU.add)
            for b in range(B):
                nc.vector.tensor_copy(PrP[:, b * 130:b * 130 + 1],
                                      PrP[:, b * 130 + 128:b * 130 + 129])
                nc.gpsimd.tensor_copy(PiP[:, b * 130:b * 130 + 1],
                                      PiP[:, b * 130 + 128:b * 130 + 129])

            # ---- step6 with OLA folded in ----
            obs = []
            for b in range(B):
                ob = trash if b == 0 else ps.tile([128, 256], F32, tag="ob1",
                                                  name="ob1")
                u = b * 130 + 1   # unshifted stationary cols; u-1 = shifted
                nc.tensor.matmul(ob[:, :], PrP[:, u:u + 128], co[:, 0:256],
                                 start=True, stop=False)
                nc.tensor.matmul(ob[:, :], PiP[:, u:u + 128], so[:, 0:256],
                                 start=False, stop=False)
                nc.tensor.matmul(ob[:, :], PrP[:, u - 1:u + 127],
                                 co[:, 256:512], start=False, stop=False)
                nc.tensor.matmul(ob[:, :], PiP[:, u - 1:u + 127],
                                 so[:, 256:512], start=False, stop=True)
                ot = sb.tile([128, 256], F16, tag=f"obs{b}", name=f"obs{b}")
                nc.scalar.copy(ot[:, :], ob[:, :])
                obs.append(ot)

            for b in range(B):
                dst = bass.AP(out_e[:, :].tensor, b * T * HOP,
                              [[HOP, T], [1, HOP]])
                eng = nc.sync if b == 0 else nc.scalar
                eng.dma_start(out=dst, in_=obs[b][:, :])

    return nc


def _get_nc():
    global _NC
    if _NC is None:
        _NC = _build_nc()
        _NC.finalize()
    return _NC


# ---------------- host orchestration ----------------
def kernel(x, z, W, b):
    global LAST_RESULT
    x = np.ascontiguousarray(np.asarray(x, dtype=np.float32))
    z = np.ascontiguousarray(np.asarray(z, dtype=np.float32))
    W = np.ascontiguousarray(np.asarray(W, dtype=np.float32))
    b = np.ascontiguousarray(np.asarray(b, dtype=np.float32))

    # xcat: 3 shifted copies of x^T + ones row -> [241, 256]
    xT = np.ascontiguousarray(x.reshape(BT, D).T)                 # [80, 256]
    xsh = np.zeros((3, D, BT), np.float32)
    xsh[1] = xT
    xv = xT.reshape(D, B, T)
    xsh[0].reshape(D, B, T)[:, :, 1:] = xv[:, :, :-1]
    xsh[2].reshape(D, B, T)[:, :, :-1] = xv[:, :, 1:]
    xcat = np.concatenate([xsh.reshape(3 * D, BT),
                           np.ones((1, BT), np.float32)], axis=0)  # [241,256]
    w2 = np.concatenate([W[:, :, 0].T, W[:, :, 1].T, W[:, :, 2].T,
                         b[None, :]], axis=0)                      # [241,222]
    G = (w2 @ _CF).astype(np.float16)                              # [241,1024]
    H = (w2 @ _SF).astype(np.float16)

    # frames^T: fr[u_low, mc*BT + b*T + t] = zpad[b, t*HOP + mc*128 + u_low]
    zpad = np.concatenate(
        [np.zeros((B, HOP), np.float32), z[:, 0, :]], axis=1)     # [2, 33024]
    fidx = (np.arange(T)[:, None] * HOP + np.arange(WIN)[None, :])
    frames = zpad[:, fidx]                                        # [B,T,WIN]
    fr = frames.reshape(B, T, 4, 128).transpose(3, 2, 0, 1) \
        .reshape(128, 4 * BT).astype(np.float16)

    xc16 = xcat.astype(np.float16)
    in_maps = []
    for c in range(NCORES):
        sl = slice(c * FS, (c + 1) * FS)
        a = np.zeros((128, CA), np.float16)
        a[0:121, 0:256] = xc16[0:121]
        a[0:120, 256:512] = xc16[121:241]
        a[0:121, 512:640] = G[0:121, sl]
        a[0:120, 640:768] = G[121:241, sl]
        a[0:121, 768:896] = H[0:121, sl]
        a[0:120, 896:1024] = H[121:241, sl]
        e = np.concatenate([fr, _ZCS[c]], axis=1)
        in_maps.append({"bigA": a, "bigD": _BIGD[c],
                        "bigE": np.ascontiguousarray(e)})

    nc = _get_nc()
    res = run_bass_kernel_spmd(nc, in_maps, list(range(NCORES)), trace=TRACE)
    LAST_RESULT = res
    out = np.zeros((B, T * HOP), dtype=np.float32)
    for r in res.results:
        out += np.asarray(r["out"], dtype=np.float32)
    return out.reshape(B, 1, T * HOP)
